# revision 15
# baseline (speedup 1.0000x reference)
"""CentroidAttention Trainium2 kernel (8 NeuronCores, SPMD data-parallel over batch).

Reference computation (per problem):
    centers = segment_mean(features, labels, C=1000)       # [C, F]
    q = features @ Wq; k = centers @ Wk; v = centers @ Wv  # [B,A],[C,A],[C,A]
    P = softmax(q @ k.T / sqrt(A))                         # [B, C]
    attn = P @ v @ Wproj + bproj                           # [B, F]
    out = concat([features, attn], -1)                     # [B, 2F]

Sharding: batch B=16384 split 8 ways (2048 rows/core). Each core computes
partial segment sums+counts (one-hot matmul, transposed layout sums.T
[F, C]), AllReduce's them (fp16), then runs the attention pipeline on its
own batch shard. Weights replicated.

fp8 strategy: logits here are tiny (sigma ~0.1), so exp(S) ~= 1 and
attn ~= mean(v) + small deviation. We decompose EXACTLY:
    e = 1 + d,  d = exp(logits) - 1            (sigma ~0.1, fp8-safe)
    attnU = V1 + D,  V1 = sum_c v_c (fp16-exact), D = sum_c d_c v_c (fp8)
    denom = 1000 + sum_c d                      (fp32 accum)
All heavy matmuls (qT, kU.T, vU, S.T, PV-deviation, out-proj-deviation)
run float8e4 with perf_mode=DoubleRow (K=256/instr, ~1.5x PE throughput).
fp8 quantization error then only enters through the deviation path
(~10x attenuated) or the logits (another ~10x attenuated); the dominant
V1 path stays fp16: total ~1.3e-2 rel err vs the 2e-2 gate (numpy sim).

Scale folds (all exact fp32 constants): weights x64 into fp8;
qT8 = 8q, kT8 = 4k, sums8 = sums/8, v8 = 8v, attnD8 = 8D.T,
out PSUM = 512*(D@Wp + V1p + bp*denom), final evict scale recipD/512.
DoubleRow operand pairs use [128, 2, N] tiles; both operands of each DR
matmul use the same k-chunk pairing, so the contraction is exact.
"""

import numpy as np

import concourse.bass as bass
import concourse.bacc as bacc
import concourse.mybir as mybir
import concourse.tile as tile
from concourse.bass_utils import run_bass_kernel_spmd
from concourse.masks import make_identity

P = 128
B_LOCAL = 2048          # batch rows per core
F = 1024                # feature dim
A = 512                 # attention dim
C = 1000                # num classes
CP = 1024               # classes padded
NB = B_LOCAL // P       # 16 batch chunks
NF = F // P             # 8 feature chunks
NA = A // P             # 4 attn-dim chunks
NCC = CP // P           # 8 class chunks
N_CORES = 8
SCALE = float(A) ** -0.5
WS = 64.0               # weight fp8 scale
DR = mybir.MatmulPerfMode.DoubleRow

F32 = mybir.dt.float32
F16 = mybir.dt.float16
F8 = mybir.dt.float8e4


def _emit(tc, collective=True, io=None, upto=10):
    nc = tc.nc
    if io is None:
        io = _declare_io(nc)
    (feat_dram, lab_dram, wq_dram, wk_dram, wv_dram, wp_dram, bp_dram,
     out_dram) = io

    from contextlib import ExitStack

    with ExitStack() as ctx:
        consts = ctx.enter_context(tc.tile_pool(name="consts", bufs=1))
        stage = ctx.enter_context(tc.tile_pool(name="stage", bufs=1))
        p1024 = ctx.enter_context(tc.tile_pool(name="p1024", bufs=1))
        big8 = ctx.enter_context(tc.tile_pool(name="big8", bufs=1))
        e16p = ctx.enter_context(tc.tile_pool(name="e16p", bufs=1))
        dram = ctx.enter_context(tc.tile_pool(name="dram", bufs=1, space="DRAM"))

        def stage_tile(name):
            return stage.tile([P, 1024], F32, name=name, tag="stage", bufs=4)

        def c1024_tile(name):
            # fp16 [128,1024]: onehots (16 live) then reduced sums (8)
            return p1024.tile([P, CP], F16, name=name, tag="c1024", bufs=16)

        def big8_tile(name):
            # fp8 pair tiles [128, 2, 2048]: featT(4), qT(2), d(4), attnD(2)
            # peak live is 6 (FT4+QT2, then D4+AT2); rotation reuses slots
            return big8.tile([P, 2, B_LOCAL], F8, name=name, tag="big8",
                             bufs=6)

        def e16_tile(name):
            return e16p.tile([P, B_LOCAL], F16, name=name, tag="e16", bufs=2)

        def pair5_tile(name):
            # fp8 pair tiles [128, 2, 512]: WQ(4), WK(4), WV8(4), V8(4)
            return pair5.tile([P, 2, A], F8, name=name, tag="p5", bufs=16)

        def pair10_tile(name):
            # fp8 pair tiles [128, 2, 1024]: SM(4), KT(2), WP(2)
            return pair10.tile([P, 2, CP], F8, name=name, tag="p10", bufs=8)

        # ---- constants ----
        identity = consts.tile([P, P], F16, name="identity")
        make_identity(nc, identity)
        one1 = consts.tile([1, 1], F32, name="one1")
        nc.gpsimd.memset(one1, 1.0)
        ones_col = consts.tile([P, 1], F16, name="ones_col")
        nc.gpsimd.memset(ones_col, 1.0)
        ones_col8 = consts.tile([P, 1], F8, name="ones_col8")
        nc.gpsimd.memset(ones_col8, 1.0)
        ones_row = consts.tile([1, P], F16, name="ones_row")
        nc.gpsimd.memset(ones_row, 1.0)
        iota_g = consts.tile([P, CP], F32, name="iota_g")
        nc.gpsimd.iota(iota_g, pattern=[[1, CP]], base=0, channel_multiplier=0,
                       allow_small_or_imprecise_dtypes=True)
        iota = consts.tile([P, CP], F32, name="iota")
        nc.vector.tensor_copy(iota, iota_g)
        labels_ld = consts.tile([P, NB], F32, name="labels_ld")
        nc.sync.dma_start(labels_ld, lab_dram)
        labels_sb = consts.tile([P, NB], F32, name="labels_sb")
        nc.vector.tensor_copy(labels_sb, labels_ld)
        # warm the ACT Exp table during the load phase
        exp_warm = consts.tile([P, 1], F32, name="exp_warm")
        nc.scalar.activation(exp_warm, labels_sb[:, 0:1],
                             mybir.ActivationFunctionType.Exp,
                             bias=0.0, scale=0.0)

        # ---- collective bounce buffers ----
        QTR = 2 * P
        bcnt_in = dram.tile([1, CP], F32, name="bcnt_in")
        bcnt_out = dram.tile([1, CP], F32, name="bcnt_out",
                             addr_space="Shared")
        bnc_in, bnc_out = [], []
        for q in range(4):
            bnc_in.append(dram.tile([QTR, CP], F16, name=f"bnc_in{q}"))
            bnc_out.append(dram.tile([QTR, CP], F16, name=f"bnc_out{q}",
                                     addr_space="Shared"))

        # ---- phase 0: load features (cast fp16) and build one-hot ----
        # feats live only through phase B; scoped pool frees their SBUF
        fb_ctx = tc.tile_pool(name="fbpool", bufs=1)
        fbpool = fb_ctx.__enter__()
        feats = []
        for k in range(NB):
            st = stage_tile(f"fst{k}")
            nc.sync.dma_start(st, feat_dram[k * P:(k + 1) * P, :])
            fb = fbpool.tile([P, F], F16, name=f"featN{k}")
            nc.scalar.copy(fb, st)
            feats.append(fb)
        onehots = []
        for k in range(NB):
            oh = c1024_tile(f"onehot{k}")
            nc.vector.tensor_scalar(oh, iota, labels_sb[:, k:k + 1], None,
                                    mybir.AluOpType.is_equal)
            onehots.append(oh)

        # ---- phase A: counts = ones.T @ onehot ----
        with tc.tile_pool(name="pcnt", bufs=1, space="PSUM") as pcnt:
            cps = pcnt.tile([1, CP], F32, name="counts_ps")
            for k in range(NB):
                for h in range(2):
                    nc.tensor.matmul(cps[:, h * 512:(h + 1) * 512],
                                     lhsT=ones_col,
                                     rhs=onehots[k][:, h * 512:(h + 1) * 512],
                                     start=(k == 0), stop=(k == NB - 1))
            cnt_sb = consts.tile([1, CP], F32, name="cnt_sb")
            nc.vector.tensor_copy(cnt_sb, cps)
            nc.sync.dma_start(bcnt_in, cnt_sb)
        if collective:
            nc.gpsimd.collective_compute(
                "AllReduce", mybir.AluOpType.add,
                replica_groups=[list(range(N_CORES))],
                ins=[bcnt_in.opt()], outs=[bcnt_out.opt()],
            )
        else:
            nc.sync.dma_start(bcnt_out, bcnt_in)

        # ---- phase B: fp16 segment sums (transposed) + feat.T (fp8 pairs) ----
        featT8 = [None] * (NF // 2)     # [128, 2, 2048] fp8 j-pairs
        with tc.tile_pool(name="pseg", bufs=1, space="PSUM") as pseg:
            for jp in range(0, NF, 2):
                sps_p, ftA_p, ftB_p = {}, {}, {}
                for j in (jp, jp + 1):
                    sps_p[j] = pseg.tile([P, CP], F32, name=f"sums{j}",
                                         tag="sums", bufs=2)
                    ftA_p[j] = pseg.tile([P, F], F16, name=f"ftA{j}",
                                         tag="ftA", bufs=2)
                    ftB_p[j] = pseg.tile([P, F], F16, name=f"ftB{j}",
                                         tag="ftB", bufs=2)
                for k in range(NB):
                    for j in (jp, jp + 1):
                        lhsT = feats[k][:, j * P:(j + 1) * P]
                        for h in range(2):
                            nc.tensor.matmul(
                                sps_p[j][:, h * 512:(h + 1) * 512],
                                lhsT=lhsT,
                                rhs=onehots[k][:, h * 512:(h + 1) * 512],
                                start=(k == 0), stop=(k == NB - 1))
                        ft = ftA_p[j] if k < 8 else ftB_p[j]
                        nc.tensor.transpose(ft[:, (k % 8) * P:(k % 8 + 1) * P],
                                            lhsT, identity)
                ftile = big8_tile(f"featT{jp // 2}")
                featT8[jp // 2] = ftile
                for j in (jp, jp + 1):
                    r = j - jp
                    nc.vector.tensor_copy(ftile[:, r, 0:F], ftA_p[j])
                    nc.vector.tensor_copy(ftile[:, r, F:2 * F], ftB_p[j])
                    sums_sb = e16p.tile([P, CP], F16, name=f"sums_f16_{j}",
                                        tag="sf16", bufs=2)
                    nc.vector.tensor_copy(sums_sb, sps_p[j])
                    nc.sync.dma_start(
                        bnc_in[jp // 2][r * P:(r + 1) * P, :], sums_sb)
                if collective:
                    nc.gpsimd.collective_compute(
                        "AllReduce", mybir.AluOpType.add,
                        replica_groups=[list(range(N_CORES))],
                        ins=[bnc_in[jp // 2].opt()],
                        outs=[bnc_out[jp // 2].opt()],
                    )
                else:
                    nc.sync.dma_start(bnc_out[jp // 2], bnc_in[jp // 2])
        fb_ctx.__exit__(None, None, None)
        if upto <= 2:
            return
        # late pools open after fbpool closes so they reuse its SBUF range
        pair5 = ctx.enter_context(tc.tile_pool(name="pair5", bufs=1))
        pair10 = ctx.enter_context(tc.tile_pool(name="pair10", bufs=1))
        w16 = ctx.enter_context(tc.tile_pool(name="w16", bufs=1))

        # ---- weights: load + scale x64 + cast fp8 pairs (and fp16 copies
        # of Wv/Wproj for the exact V1 path). Overlaps the collective. ----
        WQ, WK, WV8 = [], [], []
        wv16 = []
        for nm, src, dst in (("wq", wq_dram, WQ), ("wk", wk_dram, WK),
                             ("wv", wv_dram, WV8)):
            for jp in range(0, NF, 2):
                pt = pair5_tile(f"{nm}8_{jp // 2}")
                for j in (jp, jp + 1):
                    st = stage_tile(f"{nm}st{j}")
                    nc.sync.dma_start(st[:, 0:A], src[j * P:(j + 1) * P, :])
                    nc.vector.tensor_scalar_mul(pt[:, j - jp, :],
                                                st[:, 0:A], WS)
                    if nm == "wv":
                        wb = w16.tile([P, A], F16, name=f"wv16_{j}")
                        nc.vector.tensor_scalar_mul(wb, st[:, 0:A], WS)
                        wv16.append(wb)
                dst.append(pt)
        WP, wp16 = [], []
        for ap in range(0, NA, 2):
            pt = pair10_tile(f"wp8_{ap // 2}")
            for a in (ap, ap + 1):
                st = stage_tile(f"wpst{a}")
                nc.sync.dma_start(st, wp_dram[a * P:(a + 1) * P, :])
                nc.vector.tensor_scalar_mul(pt[:, a - ap, :], st, WS)
                wb = w16.tile([P, F], F16, name=f"wp16_{a}")
                nc.vector.tensor_scalar_mul(wb, st, WS)
                wp16.append(wb)
            WP.append(pt)
        bst = stage_tile("bst")
        nc.sync.dma_start(bst[0:1, :], bp_dram)
        bp512 = consts.tile([1, F], F16, name="bp512")
        nc.vector.tensor_scalar_mul(bp512, bst[0:1, :], 512.0)
        if upto <= 3:
            return

        # ---- q.T fp8-DR (PE busy during the collective) ----
        # PSUM = (64Wq).T @ featT = 64*qT ; evict scale 1/8 -> qT8 = 8*qT
        QT = []
        with tc.tile_pool(name="pq", bufs=1, space="PSUM") as pq:
            for ap in range(0, NA, 2):
                QT.append(big8_tile(f"qT8_{ap // 2}"))
            for a in range(NA):
                for nh in range(2):
                    qps = pq.tile([P, F], F32, name=f"qps{a}_{nh}",
                                  tag="q", bufs=4)
                    for jp in range(NF // 2):
                        for n in range(2):
                            nc.tensor.matmul(
                                qps[:, n * 512:(n + 1) * 512],
                                lhsT=WQ[jp][:, :, a * P:(a + 1) * P],
                                rhs=featT8[jp][:, :, (nh * 2 + n) * 512:
                                               (nh * 2 + n + 1) * 512],
                                start=(jp == 0), stop=(jp == NF // 2 - 1),
                                perf_mode=DR)
                    nc.scalar.activation(
                        QT[a // 2][:, a % 2, nh * F:(nh + 1) * F], qps,
                        mybir.ActivationFunctionType.Copy,
                        bias=0.0, scale=1.0 / 8.0)

        if upto <= 4:
            return
        # ---- read back reduced sums (fp16) + counts ----
        sumsb = []
        for j in range(NF):
            sb = c1024_tile(f"sumsb{j}")
            nc.sync.dma_start(sb, bnc_out[j // 2][(j % 2) * P:(j % 2 + 1) * P, :])
            sumsb.append(sb)
        counts_sb = consts.tile([1, CP], F32, name="counts_sb")
        nc.sync.dma_start(counts_sb, bcnt_out)
        # sums8 = sums/8 fp8 j-pair tiles
        SM = []
        for jp in range(0, NF, 2):
            pt = pair10_tile(f"sums8_{jp // 2}")
            for j in (jp, jp + 1):
                nc.vector.tensor_scalar_mul(pt[:, j - jp, :], sumsb[j],
                                            1.0 / 8.0)
            SM.append(pt)

        if upto <= 5:
            return
        KT, V8 = [], []
        with tc.tile_pool(name="pkv", bufs=1, space="PSUM") as pkv:
            # recip counts in class-chunk column layout
            cpsT = pkv.tile([P, NCC], F32, name="cntT")
            for c in range(NCC):
                nc.tensor.transpose(cpsT[:, c:c + 1],
                                    counts_sb[:, c * P:(c + 1) * P], one1)
            cnt_m = consts.tile([P, NCC], F32, name="cnt_m")
            nc.vector.tensor_scalar_max(cnt_m, cpsT, 1.0)
            recip_cols = consts.tile([P, NCC], F32, name="recip_cols")
            nc.vector.reciprocal(recip_cols, cnt_m)
            # exp scale: true logits = PSUM * SCALE*recip/32
            exp_scale = consts.tile([P, NCC], F32, name="exp_scale")
            nc.vector.tensor_scalar_mul(exp_scale, recip_cols, SCALE / 32.0)
            # recip as fp16 row for the V1 broadcast
            cntm_row = consts.tile([1, CP], F32, name="cntm_row")
            nc.vector.tensor_scalar_max(cntm_row, counts_sb, 1.0)
            recip_row = consts.tile([1, CP], F32, name="recip_row")
            nc.vector.reciprocal(recip_row, cntm_row)
            recip_row16 = consts.tile([1, CP], F16, name="recip_row16")
            nc.vector.tensor_copy(recip_row16, recip_row)

            # kU.T fp8-DR: PSUM = (64Wk).T @ (sums/8) = 8*kT ;
            # evict scale 1/2 -> kT8 = 4*kT  (a-pair tiles [128,2,1024])
            for ap in range(0, NA, 2):
                KT.append(pair10_tile(f"kT8_{ap // 2}"))
            for a in range(NA):
                kps = pkv.tile([P, CP], F32, name=f"kps{a}", tag="k", bufs=2)
                for jp in range(NF // 2):
                    for h in range(2):
                        nc.tensor.matmul(kps[:, h * 512:(h + 1) * 512],
                                         lhsT=WK[jp][:, :, a * P:(a + 1) * P],
                                         rhs=SM[jp][:, :, h * 512:(h + 1) * 512],
                                         start=(jp == 0),
                                         stop=(jp == NF // 2 - 1),
                                         perf_mode=DR)
                nc.scalar.activation(KT[a // 2][:, a % 2, :], kps,
                                     mybir.ActivationFunctionType.Copy,
                                     bias=0.0, scale=0.5)

            # vU fp8-DR: PSUM = (sums/8).T @ (64Wv) = 8*vU ;
            # evict per-partition scale recip -> v8 = 8*v (c-pair tiles)
            for cp in range(0, NCC, 2):
                V8.append(pair5_tile(f"v8_{cp // 2}"))
            for c in range(NCC):
                vps = pkv.tile([P, A], F32, name=f"vps{c}", tag="v", bufs=2)
                for jp in range(NF // 2):
                    nc.tensor.matmul(vps,
                                     lhsT=SM[jp][:, :, c * P:(c + 1) * P],
                                     rhs=WV8[jp],
                                     start=(jp == 0), stop=(jp == NF // 2 - 1),
                                     perf_mode=DR)
                nc.scalar.activation(V8[c // 2][:, c % 2, :], vps,
                                     mybir.ActivationFunctionType.Copy,
                                     bias=0.0, scale=recip_cols[:, c:c + 1])

        if upto <= 6:
            return
        # ---- V1 path (exact, fp16): cbar = sum_c recip_c*sums_c ;
        # V1row = cbar @ 64Wv ; V1p = 8*V1 @ 64Wp = 512*V1p ----
        v1p_sb = consts.tile([1, F], F16, name="v1p_sb")
        with tc.tile_pool(name="pv1", bufs=1, space="PSUM") as pv1:
            # broadcast recip row across partitions
            bps = pv1.tile([P, CP], F32, name="bps")
            for h in range(2):
                nc.tensor.matmul(bps[:, h * 512:(h + 1) * 512],
                                 lhsT=ones_row,
                                 rhs=recip_row16[:, h * 512:(h + 1) * 512],
                                 start=True, stop=True)
            bcast16 = consts.tile([P, CP], F16, name="bcast16")
            nc.vector.tensor_copy(bcast16, bps)
            cbar32 = consts.tile([P, NF], F32, name="cbar32")
            scr = consts.tile([P, CP], F16, name="scr16")
            for j in range(NF):
                nc.vector.tensor_tensor(scr, sumsb[j], bcast16,
                                        mybir.AluOpType.mult)
                nc.vector.tensor_reduce(cbar32[:, j:j + 1], scr,
                                        axis=mybir.AxisListType.X,
                                        op=mybir.AluOpType.add)
            cbar16 = consts.tile([P, NF], F16, name="cbar16")
            nc.vector.tensor_copy(cbar16, cbar32)
            v1ps = pv1.tile([1, A], F32, name="v1ps")
            for j in range(NF):
                nc.tensor.matmul(v1ps, lhsT=cbar16[:, j:j + 1], rhs=wv16[j],
                                 start=(j == 0), stop=(j == NF - 1))
            v1row = consts.tile([1, A], F32, name="v1row")   # 64*V1
            nc.vector.tensor_copy(v1row, v1ps)
            v1cps = pv1.tile([P, NA], F32, name="v1cps")
            for a in range(NA):
                nc.tensor.transpose(v1cps[:, a:a + 1],
                                    v1row[:, a * P:(a + 1) * P], one1)
            v1cols16 = consts.tile([P, NA], F16, name="v1cols16")
            nc.vector.tensor_scalar_mul(v1cols16, v1cps, 1.0 / 8.0)  # 8*V1
            v1pps = pv1.tile([1, F], F32, name="v1pps")
            for a in range(NA):
                for h in range(2):
                    nc.tensor.matmul(v1pps[:, h * 512:(h + 1) * 512],
                                     lhsT=v1cols16[:, a:a + 1],
                                     rhs=wp16[a][:, h * 512:(h + 1) * 512],
                                     start=(a == 0), stop=(a == NA - 1))
            nc.vector.tensor_copy(v1p_sb, v1pps)   # 512*V1p fp16

        if upto <= 7:
            return
        # ---- S.T fp8-DR + exp -> e16 -> d8 = e - 1 (fp8 c-pair tiles) ----
        D8 = []
        with tc.tile_pool(name="pst", bufs=1, space="PSUM") as pst:
            for cp in range(0, NCC, 2):
                D8.append(big8_tile(f"d8_{cp // 2}"))
            for c in range(NCC):
                rows = (C - c * P) if c == NCC - 1 else P
                dst = D8[c // 2][:, c % 2, :]
                if rows < P:
                    nc.vector.memset(dst, 0.0)
                est = e16_tile(f"e16_{c}")
                for nh in range(2):
                    sps = pst.tile([P, F], F32, name=f"stps{c}_{nh}",
                                   tag="st", bufs=4)
                    for ap in range(NA // 2):
                        for n in range(2):
                            nc.tensor.matmul(
                                sps[:, n * 512:(n + 1) * 512],
                                lhsT=KT[ap][:, :, c * P:(c + 1) * P],
                                rhs=QT[ap][:, :, (nh * 2 + n) * 512:
                                           (nh * 2 + n + 1) * 512],
                                start=(ap == 0), stop=(ap == NA // 2 - 1),
                                perf_mode=DR)
                    nc.scalar.activation(est[0:rows, nh * F:(nh + 1) * F],
                                         sps[0:rows, :],
                                         mybir.ActivationFunctionType.Exp,
                                         bias=0.0,
                                         scale=exp_scale[0:rows, c:c + 1])
                nc.vector.tensor_scalar(dst[0:rows, :], est[0:rows, :],
                                        -1.0, None, mybir.AluOpType.add)

        if upto <= 8:
            return
        # ---- attnD.T fp8-DR [A, B] = (8v).T @ d = 8*D.T, plus denom ----
        recipD512_cols = consts.tile([P, NB], F32, name="recipD512_cols")
        denom16_row = consts.tile([1, B_LOCAL], F16, name="denom16_row")
        AT = []
        with tc.tile_pool(name="ppv", bufs=1, space="PSUM") as ppv:
            dps = ppv.tile([1, B_LOCAL], F32, name="dps")
            for ap in range(0, NA, 2):
                AT.append(big8_tile(f"attnD8_{ap // 2}"))
            for a in range(NA):
                for nh in range(2):
                    aps = ppv.tile([P, F], F32, name=f"aps{a}_{nh}",
                                   tag="av", bufs=2)
                    for cp in range(NCC // 2):
                        for n in range(2):
                            nc.tensor.matmul(
                                aps[:, n * 512:(n + 1) * 512],
                                lhsT=V8[cp][:, :, a * P:(a + 1) * P],
                                rhs=D8[cp][:, :, (nh * 2 + n) * 512:
                                           (nh * 2 + n + 1) * 512],
                                start=(cp == 0), stop=(cp == NCC // 2 - 1),
                                perf_mode=DR)
                        if a == 0:
                            for n in range(2):
                                for r in range(2):
                                    nc.tensor.matmul(
                                        dps[:, (nh * 2 + n) * 512:
                                            (nh * 2 + n + 1) * 512],
                                        lhsT=ones_col8,
                                        rhs=D8[cp][:, r,
                                                   (nh * 2 + n) * 512:
                                                   (nh * 2 + n + 1) * 512],
                                        start=(cp == 0 and r == 0),
                                        stop=(cp == NCC // 2 - 1 and r == 1))
                    nc.scalar.copy(AT[a // 2][:, a % 2, nh * F:(nh + 1) * F],
                                   aps)
                if a == 0:
                    # denom = 1000 + sum_c d (in-place in PSUM); recipD = 1/d
                    nc.vector.tensor_scalar(dps, dps, 1000.0, None,
                                            mybir.AluOpType.add)
                    nc.vector.tensor_copy(denom16_row, dps)
                    recipD = consts.tile([1, B_LOCAL], F32, name="recipD")
                    nc.vector.reciprocal(recipD, dps)

        if upto <= 9:
            return
        # ---- out = (512*(D@Wp) + 512*V1p + 512*bp*denom) * recipD/512 ----
        with tc.tile_pool(name="po", bufs=1, space="PSUM") as po:
            rdps = po.tile([P, NB], F32, name="rdps")
            for t in range(NB):
                nc.tensor.transpose(rdps[:, t:t + 1],
                                    recipD[:, t * P:(t + 1) * P], one1)
            nc.vector.tensor_scalar_mul(recipD512_cols, rdps, 1.0 / 512.0)
            for t in range(NB):
                ops = po.tile([P, F], F32, name=f"ops{t}", tag="o", bufs=2)
                for h in range(2):
                    for ap in range(NA // 2):
                        nc.tensor.matmul(ops[:, h * 512:(h + 1) * 512],
                                         lhsT=AT[ap][:, :, t * P:(t + 1) * P],
                                         rhs=WP[ap][:, :, h * 512:(h + 1) * 512],
                                         start=(ap == 0), stop=False,
                                         perf_mode=DR,
                                         skip_group_check=True)
                    # + 512*V1p (ones row) and + 512*bp*denom (denom row)
                    nc.tensor.matmul(ops[:, h * 512:(h + 1) * 512],
                                     lhsT=ones_row,
                                     rhs=v1p_sb[:, h * 512:(h + 1) * 512],
                                     start=False, stop=False,
                                     skip_group_check=True)
                    nc.tensor.matmul(ops[:, h * 512:(h + 1) * 512],
                                     lhsT=denom16_row[:, t * P:(t + 1) * P],
                                     rhs=bp512[:, h * 512:(h + 1) * 512],
                                     start=False, stop=True,
                                     skip_group_check=True)
                osb = stage_tile(f"osb{t}")
                nc.scalar.activation(osb, ops,
                                     mybir.ActivationFunctionType.Copy,
                                     bias=0.0,
                                     scale=recipD512_cols[:, t:t + 1])
                nc.sync.dma_start(out_dram[t * P:(t + 1) * P, :], osb)


def _declare_io(nc):
    return (
        nc.dram_tensor("features", [B_LOCAL, F], F32, kind="ExternalInput")[:],
        nc.dram_tensor("labels_f32", [P, NB], F32, kind="ExternalInput")[:],
        nc.dram_tensor("Wq", [F, A], F32, kind="ExternalInput")[:],
        nc.dram_tensor("Wk", [F, A], F32, kind="ExternalInput")[:],
        nc.dram_tensor("Wv", [F, A], F32, kind="ExternalInput")[:],
        nc.dram_tensor("Wproj", [A, F], F32, kind="ExternalInput")[:],
        nc.dram_tensor("bproj", [1, F], F32, kind="ExternalInput")[:],
        nc.dram_tensor("out", [B_LOCAL, F], F32, kind="ExternalOutput")[:],
    )


_BUILT = {}


def _get_nc(collective=True, reps=1, upto=10):
    key = (collective, reps, upto)
    if key not in _BUILT:
        nc = bacc.Bacc("TRN2", target_bir_lowering=False, debug=False,
                       num_devices=N_CORES)
        with tile.TileContext(nc) as tc:
            io = _declare_io(nc)
            for r in range(reps):
                if r:
                    tc.strict_bb_all_engine_barrier()
                _emit(tc, collective=collective, io=io, upto=upto)
        nc.compile()
        _BUILT[key] = nc
    return _BUILT[key]


def _make_in_maps(inputs):
    features = np.ascontiguousarray(np.asarray(inputs["features"],
                                               dtype=np.float32))
    labels = np.ascontiguousarray(np.asarray(inputs["labels"])).astype(np.int64)
    Wq = np.ascontiguousarray(np.asarray(inputs["Wq"], dtype=np.float32))
    Wk = np.ascontiguousarray(np.asarray(inputs["Wk"], dtype=np.float32))
    Wv = np.ascontiguousarray(np.asarray(inputs["Wv"], dtype=np.float32))
    Wproj = np.ascontiguousarray(np.asarray(inputs["Wproj"], dtype=np.float32))
    bproj = np.ascontiguousarray(
        np.asarray(inputs["bproj"], dtype=np.float32)).reshape(1, F)

    in_maps = []
    for cix in range(N_CORES):
        fl = features[cix * B_LOCAL:(cix + 1) * B_LOCAL]
        ll = labels[cix * B_LOCAL:(cix + 1) * B_LOCAL]
        lab2d = np.ascontiguousarray(
            ll.astype(np.float32).reshape(NB, P).T)
        in_maps.append({
            "features": fl,
            "labels_f32": lab2d,
            "Wq": Wq, "Wk": Wk, "Wv": Wv, "Wproj": Wproj, "bproj": bproj,
        })
    return in_maps


def _assemble(inputs, results):
    features = np.asarray(inputs["features"], dtype=np.float32)
    out = np.empty((N_CORES * B_LOCAL, 2 * F), np.float32)
    out[:, :F] = features
    for cix in range(N_CORES):
        out[cix * B_LOCAL:(cix + 1) * B_LOCAL, F:] = results[cix]["out"]
    return out


def _run(inputs, **run_kwargs):
    nc = _get_nc()
    in_maps = _make_in_maps(inputs)
    res = run_bass_kernel_spmd(nc, in_maps, list(range(N_CORES)), **run_kwargs)
    return _assemble(inputs, res.results), res


def kernel(**inputs):
    out, _ = _run(inputs)
    return out


# revision 23
# speedup vs baseline: 3.5322x; 3.5322x over previous
"""CentroidAttention Trainium2 kernel (8 NeuronCores, SPMD data-parallel over batch).

Reference computation (per problem):
    centers = segment_mean(features, labels, C=1000)       # [C, F]
    q = features @ Wq; k = centers @ Wk; v = centers @ Wv  # [B,A],[C,A],[C,A]
    P = softmax(q @ k.T / sqrt(A))                         # [B, C]
    attn = P @ v @ Wproj + bproj                           # [B, F]
    out = concat([features, attn], -1)                     # [B, 2F]

Sharding: batch B=16384 split 8 ways (2048 rows/core). Each core computes
partial segment sums+counts (one-hot matmul, transposed layout sums.T
[F, C]), AllReduce's them (fp16), then runs the attention pipeline on its
own batch shard. Weights replicated.

fp8 strategy: logits here are tiny (sigma ~0.1), so exp(S) ~= 1 and
attn ~= mean(v) + small deviation. We decompose EXACTLY:
    e = 1 + d,  d = exp(logits) - 1            (sigma ~0.1, fp8-safe)
    attnU = V1 + D,  V1 = sum_c v_c (fp16-exact), D = sum_c d_c v_c (fp8)
    denom = 1000 + sum_c d                      (fp32 accum)
All heavy matmuls (qT, kU.T, vU, S.T, PV-deviation, out-proj-deviation)
run float8e4 with perf_mode=DoubleRow (K=256/instr, ~1.5x PE throughput).
fp8 quantization error then only enters through the deviation path
(~10x attenuated) or the logits (another ~10x attenuated); the dominant
V1 path stays fp16: total ~1.3e-2 rel err vs the 2e-2 gate (numpy sim).

Scale folds (all exact fp32 constants): weights x64 into fp8;
qT8 = 8q, kT8 = 4k, sums8 = sums/8, v8 = 8v, attnD8 = 8D.T,
out PSUM = 512*(D@Wp + V1p + bp*denom), final evict scale recipD/512.
DoubleRow operand pairs use [128, 2, N] tiles; both operands of each DR
matmul use the same k-chunk pairing, so the contraction is exact.
"""

import numpy as np

import concourse.bass as bass
import concourse.bacc as bacc
import concourse.mybir as mybir
import concourse.tile as tile
from concourse.bass_utils import run_bass_kernel_spmd
from concourse.masks import make_identity

P = 128
B_LOCAL = 2048          # batch rows per core
F = 1024                # feature dim
A = 512                 # attention dim
C = 1000                # num classes
CP = 1024               # classes padded
NB = B_LOCAL // P       # 16 batch chunks
NF = F // P             # 8 feature chunks
NA = A // P             # 4 attn-dim chunks
NCC = CP // P           # 8 class chunks
N_CORES = 8
SCALE = float(A) ** -0.5
WS = 64.0               # weight fp8 scale
DR = mybir.MatmulPerfMode.DoubleRow

F32 = mybir.dt.float32
F16 = mybir.dt.float16
F8 = mybir.dt.float8e4


def _emit(tc, collective=True, io=None, upto=10):
    nc = tc.nc
    if io is None:
        io = _declare_io(nc)
    (feat_dram, lab_dram, wq_dram, wk_dram, wv_dram, wp_dram, bp_dram,
     out_dram) = io

    from contextlib import ExitStack

    with ExitStack() as ctx:
        consts = ctx.enter_context(tc.tile_pool(name="consts", bufs=1))
        stage = ctx.enter_context(tc.tile_pool(name="stage", bufs=1))
        p1024 = ctx.enter_context(tc.tile_pool(name="p1024", bufs=1))
        big8 = ctx.enter_context(tc.tile_pool(name="big8", bufs=1))
        e16p = ctx.enter_context(tc.tile_pool(name="e16p", bufs=1))
        dram = ctx.enter_context(tc.tile_pool(name="dram", bufs=1, space="DRAM"))

        def stage_tile(name):
            return stage.tile([P, 1024], F32, name=name, tag="stage", bufs=4)

        def c1024_tile(name):
            # fp16 [128,1024]: reduced sums readback (8 live)
            return p1024.tile([P, CP], F16, name=name, tag="c1024", bufs=8)

        def big8_tile(name):
            # fp8 pair tiles [128, 2, 2048]: featT(4), qT(2), d(4), attnD(2)
            # peak live is 6 (FT4+QT2, then D4+AT2); rotation reuses slots
            return big8.tile([P, 2, B_LOCAL], F8, name=name, tag="big8",
                             bufs=6)

        def e16_tile(name):
            return e16p.tile([P, B_LOCAL], F16, name=name, tag="e16", bufs=2)

        def pair5_tile(name):
            # fp8 pair tiles [128, 2, 512]: WQ(4), WK(4), WV8(4), V8(4)
            return pair5.tile([P, 2, A], F8, name=name, tag="p5", bufs=16)

        def pair10_tile(name):
            # fp8 pair tiles [128, 2, 1024]: SM(4), KT(2), WP(2)
            return pair10.tile([P, 2, CP], F8, name=name, tag="p10", bufs=8)

        # ---- constants ----
        identity = consts.tile([P, P], F16, name="identity")
        make_identity(nc, identity)
        one1 = consts.tile([1, 1], F32, name="one1")
        nc.gpsimd.memset(one1, 1.0)
        ones_col = consts.tile([P, 1], F16, name="ones_col")
        nc.gpsimd.memset(ones_col, 1.0)
        ones_col8 = consts.tile([P, 1], F8, name="ones_col8")
        nc.gpsimd.memset(ones_col8, 1.0)
        ones_row = consts.tile([1, P], F16, name="ones_row")
        nc.gpsimd.memset(ones_row, 1.0)
        iota_g = consts.tile([P, CP], F32, name="iota_g")
        nc.gpsimd.iota(iota_g, pattern=[[1, CP]], base=0, channel_multiplier=0,
                       allow_small_or_imprecise_dtypes=True)
        iota = consts.tile([P, CP], F32, name="iota")
        nc.vector.tensor_copy(iota, iota_g)
        labels_ld = consts.tile([P, NB], F32, name="labels_ld")
        nc.sync.dma_start(labels_ld, lab_dram)
        labels_sb = consts.tile([P, NB], F32, name="labels_sb")
        nc.vector.tensor_copy(labels_sb, labels_ld)
        # warm the ACT Exp table during the load phase
        exp_warm = consts.tile([P, 1], F32, name="exp_warm")
        nc.scalar.activation(exp_warm, labels_sb[:, 0:1],
                             mybir.ActivationFunctionType.Exp,
                             bias=0.0, scale=0.0)

        # ---- collective bounce buffers ----
        QTR = 2 * P
        bcnt_in = dram.tile([1, CP], F32, name="bcnt_in")
        bcnt_out = dram.tile([1, CP], F32, name="bcnt_out",
                             addr_space="Shared")
        bnc_in, bnc_out = [], []
        for q in range(4):
            bnc_in.append(dram.tile([QTR, CP], F16, name=f"bnc_in{q}"))
            bnc_out.append(dram.tile([QTR, CP], F16, name=f"bnc_out{q}",
                                     addr_space="Shared"))
        bws_in = dram.tile([1, F], F32, name="bws_in")
        bws_out = dram.tile([1, F], F32, name="bws_out", addr_space="Shared")

        # ---- phase 0: load features (cast fp16) and build one-hot ----
        # feats live only through phase B; scoped pool frees their SBUF
        fb_ctx = tc.tile_pool(name="fbpool", bufs=1)
        fbpool = fb_ctx.__enter__()
        feats = []          # fp16 [128, F] (transposes + weighted V1 sum)
        feats8 = []         # fp8 k-pair tiles [128, 2, F] (DR segsum lhsT)
        for kp in range(NB // 2):
            feats8.append(fbpool.tile([P, 2, F], F8, name=f"f8_{kp}",
                                      tag="f8", bufs=NB // 2))
        for k in range(NB):
            st = stage_tile(f"fst{k}")
            nc.sync.dma_start(st, feat_dram[k * P:(k + 1) * P, :])
            fb = fbpool.tile([P, F], F16, name=f"featN{k}")
            nc.scalar.copy(fb, st)
            nc.gpsimd.tensor_copy(feats8[k // 2][:, k % 2, :], st)
            feats.append(fb)
        oh8 = []            # fp8 k-pair one-hots [128, 2, CP] (DR segsum rhs)
        for kp in range(NB // 2):
            oh = fbpool.tile([P, 2, CP], F8, name=f"oh8_{kp}", tag="oh8",
                             bufs=NB // 2)
            for r in range(2):
                nc.vector.tensor_scalar(oh[:, r, :], iota,
                                        labels_sb[:, 2 * kp + r:2 * kp + r + 1],
                                        None, mybir.AluOpType.is_equal)
            oh8.append(oh)

        # ---- phase A: counts = ones.T @ onehot (fp8, fp32 accum) ----
        with tc.tile_pool(name="pcnt", bufs=1, space="PSUM") as pcnt:
            cps = pcnt.tile([1, CP], F32, name="counts_ps")
            for kp in range(NB // 2):
                for r in range(2):
                    for h in range(2):
                        nc.tensor.matmul(
                            cps[:, h * 512:(h + 1) * 512],
                            lhsT=ones_col8,
                            rhs=oh8[kp][:, r, h * 512:(h + 1) * 512],
                            start=(kp == 0 and r == 0),
                            stop=(kp == NB // 2 - 1 and r == 1))
            cnt_sb = consts.tile([1, CP], F32, name="cnt_sb")
            nc.vector.tensor_copy(cnt_sb, cps)
            nc.sync.dma_start(bcnt_in, cnt_sb)
        if collective:
            nc.gpsimd.collective_compute(
                "AllReduce", mybir.AluOpType.add,
                replica_groups=[list(range(N_CORES))],
                ins=[bcnt_in.opt()], outs=[bcnt_out.opt()],
            )
        else:
            nc.sync.dma_start(bcnt_out, bcnt_in)

        # ---- phase B: fp16 segment sums (transposed) + feat.T (fp8 pairs) ----
        featT8 = [None] * (NF // 2)     # [128, 2, 2048] fp8 j-pairs
        with tc.tile_pool(name="pseg", bufs=1, space="PSUM") as pseg:
            for jp in range(0, NF, 2):
                sps_p, ftA_p, ftB_p = {}, {}, {}
                for j in (jp, jp + 1):
                    sps_p[j] = pseg.tile([P, CP], F32, name=f"sums{j}",
                                         tag="sums", bufs=2)
                    ftA_p[j] = pseg.tile([P, F], F16, name=f"ftA{j}",
                                         tag="ftA", bufs=2)
                    ftB_p[j] = pseg.tile([P, F], F16, name=f"ftB{j}",
                                         tag="ftB", bufs=2)
                for kp in range(NB // 2):
                    for j in (jp, jp + 1):
                        for h in range(2):
                            nc.tensor.matmul(
                                sps_p[j][:, h * 512:(h + 1) * 512],
                                lhsT=feats8[kp][:, :, j * P:(j + 1) * P],
                                rhs=oh8[kp][:, :, h * 512:(h + 1) * 512],
                                start=(kp == 0), stop=(kp == NB // 2 - 1),
                                perf_mode=DR)
                        for k in (2 * kp, 2 * kp + 1):
                            lhsT = feats[k][:, j * P:(j + 1) * P]
                            ft = ftA_p[j] if k < 8 else ftB_p[j]
                            nc.tensor.transpose(
                                ft[:, (k % 8) * P:(k % 8 + 1) * P],
                                lhsT, identity)
                ftile = big8_tile(f"featT{jp // 2}")
                featT8[jp // 2] = ftile
                for j in (jp, jp + 1):
                    r = j - jp
                    nc.vector.tensor_copy(ftile[:, r, 0:F], ftA_p[j])
                    nc.vector.tensor_copy(ftile[:, r, F:2 * F], ftB_p[j])
                    sums_sb = e16p.tile([P, CP], F16, name=f"sums_f16_{j}",
                                        tag="sf16", bufs=2)
                    nc.vector.tensor_copy(sums_sb, sps_p[j])
                    nc.sync.dma_start(
                        bnc_in[jp // 2][r * P:(r + 1) * P, :], sums_sb)
                if collective:
                    nc.gpsimd.collective_compute(
                        "AllReduce", mybir.AluOpType.add,
                        replica_groups=[list(range(N_CORES))],
                        ins=[bnc_in[jp // 2].opt()],
                        outs=[bnc_out[jp // 2].opt()],
                    )
                else:
                    nc.sync.dma_start(bnc_out[jp // 2], bnc_in[jp // 2])

        # ---- weighted V1 sum (needs reduced counts): w_i = recip[label_i],
        # wsum = sum_i w_i * f_i  (fp16-exact; the dominant output term) ----
        counts_sb = consts.tile([1, CP], F32, name="counts_sb")
        nc.sync.dma_start(counts_sb, bcnt_out)
        cntm_row = consts.tile([1, CP], F32, name="cntm_row")
        nc.vector.tensor_scalar_max(cntm_row, counts_sb, 1.0)
        recip_row = consts.tile([1, CP], F32, name="recip_row")
        nc.vector.reciprocal(recip_row, cntm_row)
        recip_row16 = consts.tile([1, CP], F16, name="recip_row16")
        nc.vector.tensor_copy(recip_row16, recip_row)
        with tc.tile_pool(name="pw", bufs=1, space="PSUM") as pw:
            bps = pw.tile([P, CP], F32, name="bps")
            for h in range(2):
                nc.tensor.matmul(bps[:, h * 512:(h + 1) * 512],
                                 lhsT=ones_row,
                                 rhs=recip_row16[:, h * 512:(h + 1) * 512],
                                 start=True, stop=True)
            bcast16 = consts.tile([P, CP], F16, name="bcast16")
            nc.vector.tensor_copy(bcast16, bps)
            w32 = consts.tile([P, NB], F32, name="w32")
            scr = consts.tile([P, CP], F16, name="scr16")
            for kp in range(NB // 2):
                for r in range(2):
                    nc.vector.tensor_tensor(scr, oh8[kp][:, r, :], bcast16,
                                            mybir.AluOpType.mult)
                    nc.vector.tensor_reduce(w32[:, 2 * kp + r:2 * kp + r + 1],
                                            scr, axis=mybir.AxisListType.X,
                                            op=mybir.AluOpType.add)
            w16 = consts.tile([P, NB], F16, name="w16")
            nc.vector.tensor_copy(w16, w32)
            wsps = pw.tile([1, F], F32, name="wsps")
            for k in range(NB):
                for h in range(2):
                    nc.tensor.matmul(wsps[:, h * 512:(h + 1) * 512],
                                     lhsT=w16[:, k:k + 1],
                                     rhs=feats[k][:, h * 512:(h + 1) * 512],
                                     start=(k == 0), stop=(k == NB - 1))
            wsum_sb = consts.tile([1, F], F32, name="wsum_sb")
            nc.vector.tensor_copy(wsum_sb, wsps)
            nc.sync.dma_start(bws_in, wsum_sb)
        if collective:
            nc.gpsimd.collective_compute(
                "AllReduce", mybir.AluOpType.add,
                replica_groups=[list(range(N_CORES))],
                ins=[bws_in.opt()], outs=[bws_out.opt()],
            )
        else:
            nc.sync.dma_start(bws_out, bws_in)
        fb_ctx.__exit__(None, None, None)
        if upto <= 2:
            return
        # late pools open after fbpool closes so they reuse its SBUF range
        pair5 = ctx.enter_context(tc.tile_pool(name="pair5", bufs=1))
        pair10 = ctx.enter_context(tc.tile_pool(name="pair10", bufs=1))
        w16 = ctx.enter_context(tc.tile_pool(name="w16", bufs=1))

        # ---- weights: load + scale x64 + cast fp8 pairs (and fp16 copies
        # of Wv/Wproj for the exact V1 path). Overlaps the collective. ----
        WQ, WK, WV8 = [], [], []
        wv16 = []
        for nm, src, dst in (("wq", wq_dram, WQ), ("wk", wk_dram, WK),
                             ("wv", wv_dram, WV8)):
            for jp in range(0, NF, 2):
                pt = pair5_tile(f"{nm}8_{jp // 2}")
                for j in (jp, jp + 1):
                    st = stage_tile(f"{nm}st{j}")
                    nc.sync.dma_start(st[:, 0:A], src[j * P:(j + 1) * P, :])
                    nc.vector.tensor_scalar_mul(pt[:, j - jp, :],
                                                st[:, 0:A], WS)
                    if nm == "wv":
                        wb = w16.tile([P, A], F16, name=f"wv16_{j}")
                        nc.vector.tensor_scalar_mul(wb, st[:, 0:A], WS)
                        wv16.append(wb)
                dst.append(pt)
        WP, wp16 = [], []
        for ap in range(0, NA, 2):
            pt = pair10_tile(f"wp8_{ap // 2}")
            for a in (ap, ap + 1):
                st = stage_tile(f"wpst{a}")
                nc.sync.dma_start(st, wp_dram[a * P:(a + 1) * P, :])
                nc.vector.tensor_scalar_mul(pt[:, a - ap, :], st, WS)
                wb = w16.tile([P, F], F16, name=f"wp16_{a}")
                nc.vector.tensor_scalar_mul(wb, st, WS)
                wp16.append(wb)
            WP.append(pt)
        bst = stage_tile("bst")
        nc.sync.dma_start(bst[0:1, :], bp_dram)
        bp512 = consts.tile([1, F], F16, name="bp512")
        nc.vector.tensor_scalar_mul(bp512, bst[0:1, :], 512.0)
        if upto <= 3:
            return

        # ---- q.T fp8-DR (PE busy during the collective) ----
        # PSUM = (64Wq).T @ featT = 64*qT ; evict scale 1/8 -> qT8 = 8*qT
        QT = []
        with tc.tile_pool(name="pq", bufs=1, space="PSUM") as pq:
            for ap in range(0, NA, 2):
                QT.append(big8_tile(f"qT8_{ap // 2}"))
            for a in range(NA):
                for nh in range(2):
                    qps = pq.tile([P, F], F32, name=f"qps{a}_{nh}",
                                  tag="q", bufs=4)
                    for jp in range(NF // 2):
                        for n in range(2):
                            nc.tensor.matmul(
                                qps[:, n * 512:(n + 1) * 512],
                                lhsT=WQ[jp][:, :, a * P:(a + 1) * P],
                                rhs=featT8[jp][:, :, (nh * 2 + n) * 512:
                                               (nh * 2 + n + 1) * 512],
                                start=(jp == 0), stop=(jp == NF // 2 - 1),
                                perf_mode=DR)
                    nc.scalar.activation(
                        QT[a // 2][:, a % 2, nh * F:(nh + 1) * F], qps,
                        mybir.ActivationFunctionType.Copy,
                        bias=0.0, scale=1.0 / 8.0)

        if upto <= 4:
            return
        # ---- read back reduced sums (fp16) + counts ----
        sumsb = []
        for j in range(NF):
            sb = c1024_tile(f"sumsb{j}")
            nc.sync.dma_start(sb, bnc_out[j // 2][(j % 2) * P:(j % 2 + 1) * P, :])
            sumsb.append(sb)
        # sums8 = sums/8 fp8 j-pair tiles
        SM = []
        for jp in range(0, NF, 2):
            pt = pair10_tile(f"sums8_{jp // 2}")
            for j in (jp, jp + 1):
                nc.vector.tensor_scalar_mul(pt[:, j - jp, :], sumsb[j],
                                            1.0 / 8.0)
            SM.append(pt)

        if upto <= 5:
            return
        KT, V8 = [], []
        with tc.tile_pool(name="pkv", bufs=1, space="PSUM") as pkv:
            # recip counts in class-chunk column layout
            cpsT = pkv.tile([P, NCC], F32, name="cntT")
            for c in range(NCC):
                nc.tensor.transpose(cpsT[:, c:c + 1],
                                    counts_sb[:, c * P:(c + 1) * P], one1)
            cnt_m = consts.tile([P, NCC], F32, name="cnt_m")
            nc.vector.tensor_scalar_max(cnt_m, cpsT, 1.0)
            recip_cols = consts.tile([P, NCC], F32, name="recip_cols")
            nc.vector.reciprocal(recip_cols, cnt_m)
            # exp scale: true logits = PSUM * SCALE*recip/32
            exp_scale = consts.tile([P, NCC], F32, name="exp_scale")
            nc.vector.tensor_scalar_mul(exp_scale, recip_cols, SCALE / 32.0)

            # kU.T fp8-DR: PSUM = (64Wk).T @ (sums/8) = 8*kT ;
            # evict scale 1/2 -> kT8 = 4*kT  (a-pair tiles [128,2,1024])
            for ap in range(0, NA, 2):
                KT.append(pair10_tile(f"kT8_{ap // 2}"))
            for a in range(NA):
                kps = pkv.tile([P, CP], F32, name=f"kps{a}", tag="k", bufs=2)
                for jp in range(NF // 2):
                    for h in range(2):
                        nc.tensor.matmul(kps[:, h * 512:(h + 1) * 512],
                                         lhsT=WK[jp][:, :, a * P:(a + 1) * P],
                                         rhs=SM[jp][:, :, h * 512:(h + 1) * 512],
                                         start=(jp == 0),
                                         stop=(jp == NF // 2 - 1),
                                         perf_mode=DR)
                nc.scalar.activation(KT[a // 2][:, a % 2, :], kps,
                                     mybir.ActivationFunctionType.Copy,
                                     bias=0.0, scale=0.5)

            # vU fp8-DR: PSUM = (sums/8).T @ (64Wv) = 8*vU ;
            # evict per-partition scale recip -> v8 = 8*v (c-pair tiles)
            for cp in range(0, NCC, 2):
                V8.append(pair5_tile(f"v8_{cp // 2}"))
            for c in range(NCC):
                vps = pkv.tile([P, A], F32, name=f"vps{c}", tag="v", bufs=2)
                for jp in range(NF // 2):
                    nc.tensor.matmul(vps,
                                     lhsT=SM[jp][:, :, c * P:(c + 1) * P],
                                     rhs=WV8[jp],
                                     start=(jp == 0), stop=(jp == NF // 2 - 1),
                                     perf_mode=DR)
                nc.scalar.activation(V8[c // 2][:, c % 2, :], vps,
                                     mybir.ActivationFunctionType.Copy,
                                     bias=0.0, scale=recip_cols[:, c:c + 1])

        if upto <= 6:
            return
        # ---- V1 path (exact, fp16): cbar = sum_c recip_c*sums_c ;
        # V1row = cbar @ 64Wv ; V1p = 8*V1 @ 64Wp = 512*V1p ----
        v1p_sb = consts.tile([1, F], F16, name="v1p_sb")
        with tc.tile_pool(name="pv1", bufs=1, space="PSUM") as pv1:
            # cbar (class-balanced feature sum) from the reduced wsum
            wsum_r = consts.tile([1, F], F32, name="wsum_r")
            nc.sync.dma_start(wsum_r, bws_out)
            cbps = pv1.tile([P, NF], F32, name="cbps")
            for j in range(NF):
                nc.tensor.transpose(cbps[:, j:j + 1],
                                    wsum_r[:, j * P:(j + 1) * P], one1)
            cbar16 = consts.tile([P, NF], F16, name="cbar16")
            nc.vector.tensor_copy(cbar16, cbps)
            v1ps = pv1.tile([1, A], F32, name="v1ps")
            for j in range(NF):
                nc.tensor.matmul(v1ps, lhsT=cbar16[:, j:j + 1], rhs=wv16[j],
                                 start=(j == 0), stop=(j == NF - 1))
            v1row = consts.tile([1, A], F32, name="v1row")   # 64*V1
            nc.vector.tensor_copy(v1row, v1ps)
            v1cps = pv1.tile([P, NA], F32, name="v1cps")
            for a in range(NA):
                nc.tensor.transpose(v1cps[:, a:a + 1],
                                    v1row[:, a * P:(a + 1) * P], one1)
            v1cols16 = consts.tile([P, NA], F16, name="v1cols16")
            nc.vector.tensor_scalar_mul(v1cols16, v1cps, 1.0 / 8.0)  # 8*V1
            v1pps = pv1.tile([1, F], F32, name="v1pps")
            for a in range(NA):
                for h in range(2):
                    nc.tensor.matmul(v1pps[:, h * 512:(h + 1) * 512],
                                     lhsT=v1cols16[:, a:a + 1],
                                     rhs=wp16[a][:, h * 512:(h + 1) * 512],
                                     start=(a == 0), stop=(a == NA - 1))
            nc.vector.tensor_copy(v1p_sb, v1pps)   # 512*V1p fp16

        if upto <= 7:
            return
        # ---- S.T fp8-DR + exp -> e16 -> d8 = e - 1 (fp8 c-pair tiles) ----
        D8 = []
        with tc.tile_pool(name="pst", bufs=1, space="PSUM") as pst:
            for cp in range(0, NCC, 2):
                D8.append(big8_tile(f"d8_{cp // 2}"))
            for c in range(NCC):
                rows = (C - c * P) if c == NCC - 1 else P
                dst = D8[c // 2][:, c % 2, :]
                if rows < P:
                    nc.vector.memset(dst, 0.0)
                est = e16_tile(f"e16_{c}")
                for nh in range(2):
                    sps = pst.tile([P, F], F32, name=f"stps{c}_{nh}",
                                   tag="st", bufs=4)
                    for ap in range(NA // 2):
                        for n in range(2):
                            nc.tensor.matmul(
                                sps[:, n * 512:(n + 1) * 512],
                                lhsT=KT[ap][:, :, c * P:(c + 1) * P],
                                rhs=QT[ap][:, :, (nh * 2 + n) * 512:
                                           (nh * 2 + n + 1) * 512],
                                start=(ap == 0), stop=(ap == NA // 2 - 1),
                                perf_mode=DR)
                    nc.scalar.activation(est[0:rows, nh * F:(nh + 1) * F],
                                         sps[0:rows, :],
                                         mybir.ActivationFunctionType.Exp,
                                         bias=0.0,
                                         scale=exp_scale[0:rows, c:c + 1])
                nc.vector.tensor_scalar(dst[0:rows, :], est[0:rows, :],
                                        -1.0, None, mybir.AluOpType.add)

        if upto <= 8:
            return
        # ---- attnD.T fp8-DR [A, B] = (8v).T @ d = 8*D.T, plus denom ----
        recipD512_cols = consts.tile([P, NB], F32, name="recipD512_cols")
        denom16_row = consts.tile([1, B_LOCAL], F16, name="denom16_row")
        AT = []
        with tc.tile_pool(name="ppv", bufs=1, space="PSUM") as ppv:
            dps = ppv.tile([1, B_LOCAL], F32, name="dps")
            for ap in range(0, NA, 2):
                AT.append(big8_tile(f"attnD8_{ap // 2}"))
            for a in range(NA):
                for nh in range(2):
                    aps = ppv.tile([P, F], F32, name=f"aps{a}_{nh}",
                                   tag="av", bufs=2)
                    for cp in range(NCC // 2):
                        for n in range(2):
                            nc.tensor.matmul(
                                aps[:, n * 512:(n + 1) * 512],
                                lhsT=V8[cp][:, :, a * P:(a + 1) * P],
                                rhs=D8[cp][:, :, (nh * 2 + n) * 512:
                                           (nh * 2 + n + 1) * 512],
                                start=(cp == 0), stop=(cp == NCC // 2 - 1),
                                perf_mode=DR)
                        if a == 0:
                            for n in range(2):
                                for r in range(2):
                                    nc.tensor.matmul(
                                        dps[:, (nh * 2 + n) * 512:
                                            (nh * 2 + n + 1) * 512],
                                        lhsT=ones_col8,
                                        rhs=D8[cp][:, r,
                                                   (nh * 2 + n) * 512:
                                                   (nh * 2 + n + 1) * 512],
                                        start=(cp == 0 and r == 0),
                                        stop=(cp == NCC // 2 - 1 and r == 1))
                    nc.scalar.copy(AT[a // 2][:, a % 2, nh * F:(nh + 1) * F],
                                   aps)
                if a == 0:
                    # denom = 1000 + sum_c d (in-place in PSUM); recipD = 1/d
                    nc.vector.tensor_scalar(dps, dps, 1000.0, None,
                                            mybir.AluOpType.add)
                    nc.vector.tensor_copy(denom16_row, dps)
                    recipD = consts.tile([1, B_LOCAL], F32, name="recipD")
                    nc.vector.reciprocal(recipD, dps)

        if upto <= 9:
            return
        # ---- out = (512*(D@Wp) + 512*V1p + 512*bp*denom) * recipD/512 ----
        with tc.tile_pool(name="po", bufs=1, space="PSUM") as po:
            rdps = po.tile([P, NB], F32, name="rdps")
            for t in range(NB):
                nc.tensor.transpose(rdps[:, t:t + 1],
                                    recipD[:, t * P:(t + 1) * P], one1)
            nc.vector.tensor_scalar_mul(recipD512_cols, rdps, 1.0 / 512.0)
            for t in range(NB):
                ops = po.tile([P, F], F32, name=f"ops{t}", tag="o", bufs=2)
                for h in range(2):
                    for ap in range(NA // 2):
                        nc.tensor.matmul(ops[:, h * 512:(h + 1) * 512],
                                         lhsT=AT[ap][:, :, t * P:(t + 1) * P],
                                         rhs=WP[ap][:, :, h * 512:(h + 1) * 512],
                                         start=(ap == 0), stop=False,
                                         perf_mode=DR,
                                         skip_group_check=True)
                    # + 512*V1p (ones row) and + 512*bp*denom (denom row)
                    nc.tensor.matmul(ops[:, h * 512:(h + 1) * 512],
                                     lhsT=ones_row,
                                     rhs=v1p_sb[:, h * 512:(h + 1) * 512],
                                     start=False, stop=False,
                                     skip_group_check=True)
                    nc.tensor.matmul(ops[:, h * 512:(h + 1) * 512],
                                     lhsT=denom16_row[:, t * P:(t + 1) * P],
                                     rhs=bp512[:, h * 512:(h + 1) * 512],
                                     start=False, stop=True,
                                     skip_group_check=True)
                osb = stage_tile(f"osb{t}")
                nc.scalar.activation(osb, ops,
                                     mybir.ActivationFunctionType.Copy,
                                     bias=0.0,
                                     scale=recipD512_cols[:, t:t + 1])
                nc.sync.dma_start(out_dram[t * P:(t + 1) * P, :], osb)


def _declare_io(nc):
    return (
        nc.dram_tensor("features", [B_LOCAL, F], F32, kind="ExternalInput")[:],
        nc.dram_tensor("labels_f32", [P, NB], F32, kind="ExternalInput")[:],
        nc.dram_tensor("Wq", [F, A], F32, kind="ExternalInput")[:],
        nc.dram_tensor("Wk", [F, A], F32, kind="ExternalInput")[:],
        nc.dram_tensor("Wv", [F, A], F32, kind="ExternalInput")[:],
        nc.dram_tensor("Wproj", [A, F], F32, kind="ExternalInput")[:],
        nc.dram_tensor("bproj", [1, F], F32, kind="ExternalInput")[:],
        nc.dram_tensor("out", [B_LOCAL, F], F32, kind="ExternalOutput")[:],
    )


_BUILT = {}


def _get_nc(collective=True, reps=1, upto=10):
    key = (collective, reps, upto)
    if key not in _BUILT:
        nc = bacc.Bacc("TRN2", target_bir_lowering=False, debug=False,
                       num_devices=N_CORES)
        with tile.TileContext(nc) as tc:
            io = _declare_io(nc)
            for r in range(reps):
                if r:
                    tc.strict_bb_all_engine_barrier()
                _emit(tc, collective=collective, io=io, upto=upto)
        nc.compile()
        _BUILT[key] = nc
    return _BUILT[key]


def _make_in_maps(inputs):
    features = np.ascontiguousarray(np.asarray(inputs["features"],
                                               dtype=np.float32))
    labels = np.ascontiguousarray(np.asarray(inputs["labels"])).astype(np.int64)
    Wq = np.ascontiguousarray(np.asarray(inputs["Wq"], dtype=np.float32))
    Wk = np.ascontiguousarray(np.asarray(inputs["Wk"], dtype=np.float32))
    Wv = np.ascontiguousarray(np.asarray(inputs["Wv"], dtype=np.float32))
    Wproj = np.ascontiguousarray(np.asarray(inputs["Wproj"], dtype=np.float32))
    bproj = np.ascontiguousarray(
        np.asarray(inputs["bproj"], dtype=np.float32)).reshape(1, F)

    in_maps = []
    for cix in range(N_CORES):
        fl = features[cix * B_LOCAL:(cix + 1) * B_LOCAL]
        ll = labels[cix * B_LOCAL:(cix + 1) * B_LOCAL]
        lab2d = np.ascontiguousarray(
            ll.astype(np.float32).reshape(NB, P).T)
        in_maps.append({
            "features": fl,
            "labels_f32": lab2d,
            "Wq": Wq, "Wk": Wk, "Wv": Wv, "Wproj": Wproj, "bproj": bproj,
        })
    return in_maps


def _assemble(inputs, results):
    features = np.asarray(inputs["features"], dtype=np.float32)
    out = np.empty((N_CORES * B_LOCAL, 2 * F), np.float32)
    out[:, :F] = features
    for cix in range(N_CORES):
        out[cix * B_LOCAL:(cix + 1) * B_LOCAL, F:] = results[cix]["out"]
    return out


def _run(inputs, **run_kwargs):
    nc = _get_nc()
    in_maps = _make_in_maps(inputs)
    res = run_bass_kernel_spmd(nc, in_maps, list(range(N_CORES)), **run_kwargs)
    return _assemble(inputs, res.results), res


def kernel(**inputs):
    out, _ = _run(inputs)
    return out


# revision 31
# speedup vs baseline: 16.2519x; 4.6010x over previous
"""CentroidAttention Trainium2 kernel (8 NeuronCores, SPMD data-parallel over batch).

Reference computation (per problem):
    centers = segment_mean(features, labels, C=1000)       # [C, F]
    q = features @ Wq; k = centers @ Wk; v = centers @ Wv  # [B,A],[C,A],[C,A]
    P = softmax(q @ k.T / sqrt(A))                         # [B, C]
    attn = P @ v @ Wproj + bproj                           # [B, F]
    out = concat([features, attn], -1)                     # [B, 2F]

Sharding: batch B=16384 split 8 ways (2048 rows/core). Each core computes
partial segment sums+counts (one-hot matmul, transposed layout sums.T
[F, C]), AllReduce's them (fp16), then runs the attention pipeline on its
own batch shard. Weights replicated.

fp8 strategy: logits here are tiny (sigma ~0.1), so exp(S) ~= 1 and
attn ~= mean(v) + small deviation. We decompose EXACTLY:
    e = 1 + d,  d = exp(logits) - 1            (sigma ~0.1, fp8-safe)
    attnU = V1 + D,  V1 = sum_c v_c (fp16-exact), D = sum_c d_c v_c (fp8)
    denom = 1000 + sum_c d                      (fp32 accum)
All heavy matmuls (qT, kU.T, vU, S.T, PV-deviation, out-proj-deviation)
run float8e4 with perf_mode=DoubleRow (K=256/instr, ~1.5x PE throughput).
fp8 quantization error then only enters through the deviation path
(~10x attenuated) or the logits (another ~10x attenuated); the dominant
V1 path stays fp16: total ~1.3e-2 rel err vs the 2e-2 gate (numpy sim).

Scale folds (all exact fp32 constants): weights x64 into fp8;
qT8 = 8q, kT8 = 4k, sums8 = sums/8, v8 = 8v, attnD8 = 8D.T,
out PSUM = 512*(D@Wp + V1p + bp*denom), final evict scale recipD/512.
DoubleRow operand pairs use [128, 2, N] tiles; both operands of each DR
matmul use the same k-chunk pairing, so the contraction is exact.
"""

import numpy as np

import concourse.bass as bass
import concourse.bacc as bacc
import concourse.mybir as mybir
import concourse.tile as tile
from concourse.bass_utils import run_bass_kernel_spmd
from concourse.masks import make_identity

P = 128
B_LOCAL = 2048          # batch rows per core
F = 1024                # feature dim
A = 512                 # attention dim
C = 1000                # num classes
CP = 1024               # classes padded
NB = B_LOCAL // P       # 16 batch chunks
NF = F // P             # 8 feature chunks
NA = A // P             # 4 attn-dim chunks
NCC = CP // P           # 8 class chunks
N_CORES = 8
SCALE = float(A) ** -0.5
WS = 64.0               # weight fp8 scale
DR = mybir.MatmulPerfMode.DoubleRow

F32 = mybir.dt.float32
F16 = mybir.dt.float16
F8 = mybir.dt.float8e4


def _emit(tc, collective=True, io=None, upto=10):
    nc = tc.nc
    if io is None:
        io = _declare_io(nc)
    (feat_dram, lab_dram, wq_dram, wk_dram, wv_dram, wp_dram, bp_dram,
     out_dram) = io

    from contextlib import ExitStack

    with ExitStack() as ctx:
        consts = ctx.enter_context(tc.tile_pool(name="consts", bufs=1))
        stage = ctx.enter_context(tc.tile_pool(name="stage", bufs=1))
        p1024 = ctx.enter_context(tc.tile_pool(name="p1024", bufs=1))
        big8 = ctx.enter_context(tc.tile_pool(name="big8", bufs=1))
        e16p = ctx.enter_context(tc.tile_pool(name="e16p", bufs=1))
        dram = ctx.enter_context(tc.tile_pool(name="dram", bufs=1, space="DRAM"))

        def stage_tile(name):
            return stage.tile([P, 1024], F32, name=name, tag="stage", bufs=4)

        def c1024_tile(name):
            # fp16 [128,1024]: reduced sums readback (8 live)
            return p1024.tile([P, CP], F16, name=name, tag="c1024", bufs=8)

        def big8_tile(name):
            # fp8 pair tiles [128, 2, 2048]: featT(4), qT(2), d(4), attnD(2)
            # peak live is 6 (FT4+QT2, then D4+AT2); rotation reuses slots
            return big8.tile([P, 2, B_LOCAL], F8, name=name, tag="big8",
                             bufs=6)

        def e16_tile(name):
            return e16p.tile([P, B_LOCAL], F16, name=name, tag="e16", bufs=2)

        def pair5_tile(name):
            # fp8 pair tiles [128, 2, 512]: WQ(4), WK(4), WV8(4), V8(4)
            return pair5.tile([P, 2, A], F8, name=name, tag="p5", bufs=16)

        def pair10_tile(name):
            # fp8 pair tiles [128, 2, 1024]: SM(4), KT(2), WP(2)
            return pair10.tile([P, 2, CP], F8, name=name, tag="p10", bufs=8)

        # ---- constants ----
        identity = consts.tile([P, P], F16, name="identity")
        make_identity(nc, identity)
        one1 = consts.tile([1, 1], F32, name="one1")
        nc.gpsimd.memset(one1, 1.0)
        ones_col = consts.tile([P, 1], F16, name="ones_col")
        nc.gpsimd.memset(ones_col, 1.0)
        ones_col8 = consts.tile([P, 1], F8, name="ones_col8")
        nc.gpsimd.memset(ones_col8, 1.0)
        ones16p8 = consts.tile([P, 2, 16], F8, name="ones16p8")
        nc.gpsimd.memset(ones16p8, 1.0)
        ones_row = consts.tile([1, P], F16, name="ones_row")
        nc.gpsimd.memset(ones_row, 1.0)
        iota_g = consts.tile([P, CP], F32, name="iota_g")
        nc.gpsimd.iota(iota_g, pattern=[[1, CP]], base=0, channel_multiplier=0,
                       allow_small_or_imprecise_dtypes=True)
        iota = consts.tile([P, CP], F32, name="iota")
        nc.vector.tensor_copy(iota, iota_g)
        labels_ld = consts.tile([P, NB], F32, name="labels_ld")
        nc.sync.dma_start(labels_ld, lab_dram)
        labels_sb = consts.tile([P, NB], F32, name="labels_sb")
        nc.vector.tensor_copy(labels_sb, labels_ld)
        # warm the ACT Exp table during the load phase
        exp_warm = consts.tile([P, 1], F32, name="exp_warm")
        nc.scalar.activation(exp_warm, labels_sb[:, 0:1],
                             mybir.ActivationFunctionType.Exp,
                             bias=0.0, scale=0.0)

        # ---- collective bounce buffers ----
        QTR = 2 * P
        bcnt_in = dram.tile([1, CP], F32, name="bcnt_in")
        bcnt_out = dram.tile([1, CP], F32, name="bcnt_out",
                             addr_space="Shared")
        bnc_in, bnc_out = [], []
        for q in range(4):
            bnc_in.append(dram.tile([QTR, CP], F16, name=f"bnc_in{q}"))
            bnc_out.append(dram.tile([QTR, CP], F16, name=f"bnc_out{q}",
                                     addr_space="Shared"))
        bws_in = dram.tile([1, F], F32, name="bws_in")
        bws_out = dram.tile([1, F], F32, name="bws_out", addr_space="Shared")

        # ---- phase 0: load features (cast fp16) and build one-hot ----
        # feats live only through phase B; scoped pool frees their SBUF
        fb_ctx = tc.tile_pool(name="fbpool", bufs=1)
        fbpool = fb_ctx.__enter__()
        feats = []          # fp16 [128, F] (transposes + weighted V1 sum)
        feats8 = []         # fp8 k-pair tiles [128, 2, F] (DR segsum lhsT)
        for kp in range(NB // 2):
            feats8.append(fbpool.tile([P, 2, F], F8, name=f"f8_{kp}",
                                      tag="f8", bufs=NB // 2))
        for k in range(NB):
            st = stage_tile(f"fst{k}")
            nc.sync.dma_start(st, feat_dram[k * P:(k + 1) * P, :])
            fb = fbpool.tile([P, F], F16, name=f"featN{k}")
            nc.scalar.copy(fb, st)
            nc.gpsimd.tensor_copy(feats8[k // 2][:, k % 2, :], st)
            feats.append(fb)
        oh8 = []            # fp8 k-pair one-hots [128, 2, CP] (DR segsum rhs)
        for kp in range(NB // 2):
            oh = fbpool.tile([P, 2, CP], F8, name=f"oh8_{kp}", tag="oh8",
                             bufs=NB // 2)
            for r in range(2):
                nc.vector.tensor_scalar(oh[:, r, :], iota,
                                        labels_sb[:, 2 * kp + r:2 * kp + r + 1],
                                        None, mybir.AluOpType.is_equal)
            oh8.append(oh)

        # ---- phase A: counts = ones.T @ onehot (fp8-DR, M=16 ones) ----
        with tc.tile_pool(name="pcnt", bufs=1, space="PSUM") as pcnt:
            cps = pcnt.tile([16, CP], F32, name="counts_ps")
            for kp in range(NB // 2):
                for h in range(2):
                    nc.tensor.matmul(
                        cps[:, h * 512:(h + 1) * 512],
                        lhsT=ones16p8,
                        rhs=oh8[kp][:, :, h * 512:(h + 1) * 512],
                        start=(kp == 0), stop=(kp == NB // 2 - 1),
                        perf_mode=DR)
            cnt_sb = consts.tile([1, CP], F32, name="cnt_sb")
            nc.vector.tensor_copy(cnt_sb, cps[0:1, :])
            nc.sync.dma_start(bcnt_in, cnt_sb)
        if collective:
            nc.gpsimd.collective_compute(
                "AllReduce", mybir.AluOpType.add,
                replica_groups=[list(range(N_CORES))],
                ins=[bcnt_in.opt()], outs=[bcnt_out.opt()],
            )
        else:
            nc.sync.dma_start(bcnt_out, bcnt_in)

        # ---- phase B: fp16 segment sums (transposed) + feat.T (fp8 pairs) ----
        featT8 = [None] * (NF // 2)     # [128, 2, 2048] fp8 j-pairs
        with tc.tile_pool(name="pseg", bufs=1, space="PSUM") as pseg:
            for jp in range(0, NF, 2):
                sps_p, ftA_p, ftB_p = {}, {}, {}
                for j in (jp, jp + 1):
                    sps_p[j] = pseg.tile([P, CP], F32, name=f"sums{j}",
                                         tag="sums", bufs=2)
                    ftA_p[j] = pseg.tile([P, F], F16, name=f"ftA{j}",
                                         tag="ftA", bufs=2)
                    ftB_p[j] = pseg.tile([P, F], F16, name=f"ftB{j}",
                                         tag="ftB", bufs=2)
                for kp in range(NB // 2):
                    for j in (jp, jp + 1):
                        for h in range(2):
                            nc.tensor.matmul(
                                sps_p[j][:, h * 512:(h + 1) * 512],
                                lhsT=feats8[kp][:, :, j * P:(j + 1) * P],
                                rhs=oh8[kp][:, :, h * 512:(h + 1) * 512],
                                start=(kp == 0), stop=(kp == NB // 2 - 1),
                                perf_mode=DR)
                        for k in (2 * kp, 2 * kp + 1):
                            lhsT = feats[k][:, j * P:(j + 1) * P]
                            ft = ftA_p[j] if k < 8 else ftB_p[j]
                            nc.tensor.transpose(
                                ft[:, (k % 8) * P:(k % 8 + 1) * P],
                                lhsT, identity)
                ftile = big8_tile(f"featT{jp // 2}")
                featT8[jp // 2] = ftile
                for j in (jp, jp + 1):
                    r = j - jp
                    nc.vector.tensor_copy(ftile[:, r, 0:F], ftA_p[j])
                    nc.vector.tensor_copy(ftile[:, r, F:2 * F], ftB_p[j])
                    sums_sb = e16p.tile([P, CP], F16, name=f"sums_f16_{j}",
                                        tag="sf16", bufs=2)
                    nc.vector.tensor_copy(sums_sb, sps_p[j])
                    nc.sync.dma_start(
                        bnc_in[jp // 2][r * P:(r + 1) * P, :], sums_sb)
                if collective:
                    nc.gpsimd.collective_compute(
                        "AllReduce", mybir.AluOpType.add,
                        replica_groups=[list(range(N_CORES))],
                        ins=[bnc_in[jp // 2].opt()],
                        outs=[bnc_out[jp // 2].opt()],
                    )
                else:
                    nc.sync.dma_start(bnc_out[jp // 2], bnc_in[jp // 2])

        # ---- weighted V1 sum (needs reduced counts): w_i = recip[label_i],
        # wsum = sum_i w_i * f_i  (fp16-exact; the dominant output term) ----
        counts_sb = consts.tile([1, CP], F32, name="counts_sb")
        nc.sync.dma_start(counts_sb, bcnt_out)
        cntm_row = consts.tile([1, CP], F32, name="cntm_row")
        nc.vector.tensor_scalar_max(cntm_row, counts_sb, 1.0)
        recip_row = consts.tile([1, CP], F32, name="recip_row")
        nc.vector.reciprocal(recip_row, cntm_row)
        recip_row16 = consts.tile([1, CP], F16, name="recip_row16")
        nc.vector.tensor_copy(recip_row16, recip_row)
        with tc.tile_pool(name="pw", bufs=1, space="PSUM") as pw:
            bps = pw.tile([P, CP], F32, name="bps")
            for h in range(2):
                nc.tensor.matmul(bps[:, h * 512:(h + 1) * 512],
                                 lhsT=ones_row,
                                 rhs=recip_row16[:, h * 512:(h + 1) * 512],
                                 start=True, stop=True)
            bcast16 = consts.tile([P, CP], F16, name="bcast16")
            nc.vector.tensor_copy(bcast16, bps)
            w32 = consts.tile([P, NB], F32, name="w32")
            scr = consts.tile([P, CP], F16, name="scr16")
            for kp in range(NB // 2):
                for r in range(2):
                    nc.vector.tensor_tensor(scr, oh8[kp][:, r, :], bcast16,
                                            mybir.AluOpType.mult)
                    nc.vector.tensor_reduce(w32[:, 2 * kp + r:2 * kp + r + 1],
                                            scr, axis=mybir.AxisListType.X,
                                            op=mybir.AluOpType.add)
            w16 = consts.tile([P, NB], F16, name="w16")
            nc.vector.tensor_copy(w16, w32)
            wsps = pw.tile([1, F], F32, name="wsps")
            for k in range(NB):
                for h in range(2):
                    nc.tensor.matmul(wsps[:, h * 512:(h + 1) * 512],
                                     lhsT=w16[:, k:k + 1],
                                     rhs=feats[k][:, h * 512:(h + 1) * 512],
                                     start=(k == 0), stop=(k == NB - 1))
            wsum_sb = consts.tile([1, F], F32, name="wsum_sb")
            nc.vector.tensor_copy(wsum_sb, wsps)
            nc.sync.dma_start(bws_in, wsum_sb)
        if collective:
            nc.gpsimd.collective_compute(
                "AllReduce", mybir.AluOpType.add,
                replica_groups=[list(range(N_CORES))],
                ins=[bws_in.opt()], outs=[bws_out.opt()],
            )
        else:
            nc.sync.dma_start(bws_out, bws_in)
        fb_ctx.__exit__(None, None, None)
        if upto <= 2:
            return
        # late pools open after fbpool closes so they reuse its SBUF range
        pair5 = ctx.enter_context(tc.tile_pool(name="pair5", bufs=1))
        pair10 = ctx.enter_context(tc.tile_pool(name="pair10", bufs=1))
        w16 = ctx.enter_context(tc.tile_pool(name="w16", bufs=1))

        # ---- weights: load + scale x64 + cast fp8 pairs (and fp16 copies
        # of Wv/Wproj for the exact V1 path). Overlaps the collective. ----
        WQ, WK, WV8 = [], [], []
        wv16 = []
        for nm, src, dst in (("wq", wq_dram, WQ), ("wk", wk_dram, WK),
                             ("wv", wv_dram, WV8)):
            for jp in range(0, NF, 2):
                pt = pair5_tile(f"{nm}8_{jp // 2}")
                for j in (jp, jp + 1):
                    st = stage_tile(f"{nm}st{j}")
                    nc.sync.dma_start(st[:, 0:A], src[j * P:(j + 1) * P, :])
                    nc.vector.tensor_scalar_mul(pt[:, j - jp, :],
                                                st[:, 0:A], WS)
                    if nm == "wv":
                        wb = w16.tile([P, A], F16, name=f"wv16_{j}")
                        nc.vector.tensor_scalar_mul(wb, st[:, 0:A], WS)
                        wv16.append(wb)
                dst.append(pt)
        WP, wp16 = [], []
        for ap in range(0, NA, 2):
            pt = pair10_tile(f"wp8_{ap // 2}")
            for a in (ap, ap + 1):
                st = stage_tile(f"wpst{a}")
                nc.sync.dma_start(st, wp_dram[a * P:(a + 1) * P, :])
                nc.vector.tensor_scalar_mul(pt[:, a - ap, :], st, WS)
                wb = w16.tile([P, F], F16, name=f"wp16_{a}")
                nc.vector.tensor_scalar_mul(wb, st, WS)
                wp16.append(wb)
            WP.append(pt)
        bst = stage_tile("bst")
        nc.sync.dma_start(bst[0:1, :], bp_dram)
        bp512 = consts.tile([1, F], F16, name="bp512")
        nc.vector.tensor_scalar_mul(bp512, bst[0:1, :], 512.0)
        if upto <= 3:
            return

        # ---- q.T fp8-DR (PE busy during the collective) ----
        # PSUM = (64Wq).T @ featT = 64*qT ; evict scale 1/8 -> qT8 = 8*qT
        QT = []
        with tc.tile_pool(name="pq", bufs=1, space="PSUM") as pq:
            for ap in range(0, NA, 2):
                QT.append(big8_tile(f"qT8_{ap // 2}"))
            for a in range(NA):
                for nh in range(2):
                    qps = pq.tile([P, F], F32, name=f"qps{a}_{nh}",
                                  tag="q", bufs=4)
                    for jp in range(NF // 2):
                        for n in range(2):
                            nc.tensor.matmul(
                                qps[:, n * 512:(n + 1) * 512],
                                lhsT=WQ[jp][:, :, a * P:(a + 1) * P],
                                rhs=featT8[jp][:, :, (nh * 2 + n) * 512:
                                               (nh * 2 + n + 1) * 512],
                                start=(jp == 0), stop=(jp == NF // 2 - 1),
                                perf_mode=DR)
                    nc.scalar.activation(
                        QT[a // 2][:, a % 2, nh * F:(nh + 1) * F], qps,
                        mybir.ActivationFunctionType.Copy,
                        bias=0.0, scale=1.0 / 8.0)

        if upto <= 4:
            return
        # ---- read back reduced sums (fp16) + counts ----
        sumsb = []
        for j in range(NF):
            sb = c1024_tile(f"sumsb{j}")
            nc.sync.dma_start(sb, bnc_out[j // 2][(j % 2) * P:(j % 2 + 1) * P, :])
            sumsb.append(sb)
        # sums8 = sums/8 fp8 j-pair tiles
        SM = []
        for jp in range(0, NF, 2):
            pt = pair10_tile(f"sums8_{jp // 2}")
            for j in (jp, jp + 1):
                nc.vector.tensor_scalar_mul(pt[:, j - jp, :], sumsb[j],
                                            1.0 / 8.0)
            SM.append(pt)

        if upto <= 5:
            return
        KT, V8 = [], []
        with tc.tile_pool(name="pkv", bufs=1, space="PSUM") as pkv:
            # recip counts in class-chunk column layout
            cpsT = pkv.tile([P, NCC], F32, name="cntT")
            for c in range(NCC):
                nc.tensor.transpose(cpsT[:, c:c + 1],
                                    counts_sb[:, c * P:(c + 1) * P], one1)
            cnt_m = consts.tile([P, NCC], F32, name="cnt_m")
            nc.vector.tensor_scalar_max(cnt_m, cpsT, 1.0)
            recip_cols = consts.tile([P, NCC], F32, name="recip_cols")
            nc.vector.reciprocal(recip_cols, cnt_m)
            # exp scale: true logits = PSUM * SCALE*recip/32
            exp_scale = consts.tile([P, NCC], F32, name="exp_scale")
            nc.vector.tensor_scalar_mul(exp_scale, recip_cols, SCALE / 32.0)

            # kU.T fp8-DR: PSUM = (64Wk).T @ (sums/8) = 8*kT ;
            # evict scale 1/2 -> kT8 = 4*kT  (a-pair tiles [128,2,1024])
            for ap in range(0, NA, 2):
                KT.append(pair10_tile(f"kT8_{ap // 2}"))
            for a in range(NA):
                kps = pkv.tile([P, CP], F32, name=f"kps{a}", tag="k", bufs=2)
                for jp in range(NF // 2):
                    for h in range(2):
                        nc.tensor.matmul(kps[:, h * 512:(h + 1) * 512],
                                         lhsT=WK[jp][:, :, a * P:(a + 1) * P],
                                         rhs=SM[jp][:, :, h * 512:(h + 1) * 512],
                                         start=(jp == 0),
                                         stop=(jp == NF // 2 - 1),
                                         perf_mode=DR)
                nc.scalar.activation(KT[a // 2][:, a % 2, :], kps,
                                     mybir.ActivationFunctionType.Copy,
                                     bias=0.0, scale=0.5)

            # vU fp8-DR: PSUM = (sums/8).T @ (64Wv) = 8*vU ;
            # evict per-partition scale recip -> v8 = 8*v (c-pair tiles)
            for cp in range(0, NCC, 2):
                V8.append(pair5_tile(f"v8_{cp // 2}"))
            for c in range(NCC):
                vps = pkv.tile([P, A], F32, name=f"vps{c}", tag="v", bufs=2)
                for jp in range(NF // 2):
                    nc.tensor.matmul(vps,
                                     lhsT=SM[jp][:, :, c * P:(c + 1) * P],
                                     rhs=WV8[jp],
                                     start=(jp == 0), stop=(jp == NF // 2 - 1),
                                     perf_mode=DR)
                nc.scalar.activation(V8[c // 2][:, c % 2, :], vps,
                                     mybir.ActivationFunctionType.Copy,
                                     bias=0.0, scale=recip_cols[:, c:c + 1])

        if upto <= 6:
            return
        # ---- V1 path (exact, fp16): cbar = sum_c recip_c*sums_c ;
        # V1row = cbar @ 64Wv ; V1p = 8*V1 @ 64Wp = 512*V1p ----
        v1p_sb = consts.tile([1, F], F16, name="v1p_sb")
        with tc.tile_pool(name="pv1", bufs=1, space="PSUM") as pv1:
            # cbar (class-balanced feature sum) from the reduced wsum
            wsum_r = consts.tile([1, F], F32, name="wsum_r")
            nc.sync.dma_start(wsum_r, bws_out)
            cbps = pv1.tile([P, NF], F32, name="cbps")
            for j in range(NF):
                nc.tensor.transpose(cbps[:, j:j + 1],
                                    wsum_r[:, j * P:(j + 1) * P], one1)
            cbar16 = consts.tile([P, NF], F16, name="cbar16")
            nc.vector.tensor_copy(cbar16, cbps)
            v1ps = pv1.tile([1, A], F32, name="v1ps")
            for j in range(NF):
                nc.tensor.matmul(v1ps, lhsT=cbar16[:, j:j + 1], rhs=wv16[j],
                                 start=(j == 0), stop=(j == NF - 1))
            v1row = consts.tile([1, A], F32, name="v1row")   # 64*V1
            nc.vector.tensor_copy(v1row, v1ps)
            v1cps = pv1.tile([P, NA], F32, name="v1cps")
            for a in range(NA):
                nc.tensor.transpose(v1cps[:, a:a + 1],
                                    v1row[:, a * P:(a + 1) * P], one1)
            v1cols16 = consts.tile([P, NA], F16, name="v1cols16")
            nc.vector.tensor_scalar_mul(v1cols16, v1cps, 1.0 / 8.0)  # 8*V1
            v1pps = pv1.tile([1, F], F32, name="v1pps")
            for a in range(NA):
                for h in range(2):
                    nc.tensor.matmul(v1pps[:, h * 512:(h + 1) * 512],
                                     lhsT=v1cols16[:, a:a + 1],
                                     rhs=wp16[a][:, h * 512:(h + 1) * 512],
                                     start=(a == 0), stop=(a == NA - 1))
            nc.vector.tensor_copy(v1p_sb, v1pps)   # 512*V1p fp16
            # broadcast true V1p and bproj to [128, F] for the STT evict
            bcps = pv1.tile([P, F], F32, name="bcps")
            for h in range(2):
                nc.tensor.matmul(bcps[:, h * 512:(h + 1) * 512],
                                 lhsT=ones_row,
                                 rhs=v1p_sb[:, h * 512:(h + 1) * 512],
                                 start=True, stop=True)
            v1pb = consts.tile([P, F], F32, name="v1pb")
            nc.vector.tensor_scalar_mul(v1pb, bcps, 1.0 / 512.0)
            for h in range(2):
                nc.tensor.matmul(bcps[:, h * 512:(h + 1) * 512],
                                 lhsT=ones_row,
                                 rhs=bp512[:, h * 512:(h + 1) * 512],
                                 start=True, stop=True)
            bpb = consts.tile([P, F], F32, name="bpb")
            nc.vector.tensor_scalar_mul(bpb, bcps, 1.0 / 512.0)

        if upto <= 7:
            return
        # ---- S.T fp8-DR + exp -> e16 -> d8 = e - 1 (fp8 c-pair tiles) ----
        D8 = []
        with tc.tile_pool(name="pst", bufs=1, space="PSUM") as pst:
            for cp in range(0, NCC, 2):
                D8.append(big8_tile(f"d8_{cp // 2}"))
            for c in range(NCC):
                rows = (C - c * P) if c == NCC - 1 else P
                dst = D8[c // 2][:, c % 2, :]
                if rows < P:
                    nc.vector.memset(dst, 0.0)
                est = e16_tile(f"e16_{c}")
                for nh in range(2):
                    sps = pst.tile([P, F], F32, name=f"stps{c}_{nh}",
                                   tag="st", bufs=4)
                    for ap in range(NA // 2):
                        for n in range(2):
                            nc.tensor.matmul(
                                sps[:, n * 512:(n + 1) * 512],
                                lhsT=KT[ap][:, :, c * P:(c + 1) * P],
                                rhs=QT[ap][:, :, (nh * 2 + n) * 512:
                                           (nh * 2 + n + 1) * 512],
                                start=(ap == 0), stop=(ap == NA // 2 - 1),
                                perf_mode=DR)
                    nc.scalar.activation(est[0:rows, nh * F:(nh + 1) * F],
                                         sps[0:rows, :],
                                         mybir.ActivationFunctionType.Exp,
                                         bias=0.0,
                                         scale=exp_scale[0:rows, c:c + 1])
                nc.vector.tensor_scalar(dst[0:rows, :], est[0:rows, :],
                                        -1.0, None, mybir.AluOpType.add)

        if upto <= 8:
            return
        # ---- attnD.T fp8-DR [A, B] = (8v).T @ d = 8*D.T, plus denom ----
        recipD512_cols = consts.tile([P, NB], F32, name="recipD512_cols")
        AT = []
        with tc.tile_pool(name="ppv", bufs=1, space="PSUM") as ppv:
            dps = ppv.tile([16, B_LOCAL], F32, name="dps")
            for ap in range(0, NA, 2):
                AT.append(big8_tile(f"attnD8_{ap // 2}"))
            for a in range(NA):
                for nh in range(2):
                    aps = ppv.tile([P, F], F32, name=f"aps{a}_{nh}",
                                   tag="av", bufs=2)
                    for cp in range(NCC // 2):
                        for n in range(2):
                            nc.tensor.matmul(
                                aps[:, n * 512:(n + 1) * 512],
                                lhsT=V8[cp][:, :, a * P:(a + 1) * P],
                                rhs=D8[cp][:, :, (nh * 2 + n) * 512:
                                           (nh * 2 + n + 1) * 512],
                                start=(cp == 0), stop=(cp == NCC // 2 - 1),
                                perf_mode=DR)
                        if a == 0:
                            for n in range(2):
                                nc.tensor.matmul(
                                    dps[:, (nh * 2 + n) * 512:
                                        (nh * 2 + n + 1) * 512],
                                    lhsT=ones16p8,
                                    rhs=D8[cp][:, :,
                                               (nh * 2 + n) * 512:
                                               (nh * 2 + n + 1) * 512],
                                    start=(cp == 0),
                                    stop=(cp == NCC // 2 - 1),
                                    perf_mode=DR)
                    nc.scalar.copy(AT[a // 2][:, a % 2, nh * F:(nh + 1) * F],
                                   aps)
                if a == 0:
                    # denom = 1000 + sum_c d (row 0 of the M=16 ones block)
                    nc.vector.tensor_scalar(dps[0:1, :], dps[0:1, :], 1000.0,
                                            None, mybir.AluOpType.add)
                    recipD = consts.tile([1, B_LOCAL], F32, name="recipD")
                    nc.vector.reciprocal(recipD, dps[0:1, :])

        if upto <= 9:
            return
        # ---- out = (512*(D@Wp)) * recipD/512 + V1p*recipD + bp ----
        with tc.tile_pool(name="po", bufs=1, space="PSUM") as po:
            rdps = po.tile([P, NB], F32, name="rdps")
            for t in range(NB):
                nc.tensor.transpose(rdps[:, t:t + 1],
                                    recipD[:, t * P:(t + 1) * P], one1)
            recipD_cols = consts.tile([P, NB], F32, name="recipD_cols")
            nc.vector.tensor_copy(recipD_cols, rdps)
            nc.vector.tensor_scalar_mul(recipD512_cols, recipD_cols,
                                        1.0 / 512.0)
            for t in range(NB):
                ops = po.tile([P, F], F32, name=f"ops{t}", tag="o", bufs=2)
                for h in range(2):
                    for ap in range(NA // 2):
                        nc.tensor.matmul(ops[:, h * 512:(h + 1) * 512],
                                         lhsT=AT[ap][:, :, t * P:(t + 1) * P],
                                         rhs=WP[ap][:, :, h * 512:(h + 1) * 512],
                                         start=(ap == 0),
                                         stop=(ap == NA // 2 - 1),
                                         perf_mode=DR)
                # tmp = V1p*recipD + bp ; osb = ops*recipD/512 + tmp
                tmp = stage_tile(f"otmp{t}")
                nc.vector.scalar_tensor_tensor(
                    tmp, v1pb, recipD_cols[:, t:t + 1], bpb,
                    op0=mybir.AluOpType.mult, op1=mybir.AluOpType.add)
                osb = stage_tile(f"osb{t}")
                nc.vector.scalar_tensor_tensor(
                    osb, ops, recipD512_cols[:, t:t + 1], tmp,
                    op0=mybir.AluOpType.mult, op1=mybir.AluOpType.add)
                nc.sync.dma_start(out_dram[t * P:(t + 1) * P, :], osb)


def _declare_io(nc):
    return (
        nc.dram_tensor("features", [B_LOCAL, F], F32, kind="ExternalInput")[:],
        nc.dram_tensor("labels_f32", [P, NB], F32, kind="ExternalInput")[:],
        nc.dram_tensor("Wq", [F, A], F32, kind="ExternalInput")[:],
        nc.dram_tensor("Wk", [F, A], F32, kind="ExternalInput")[:],
        nc.dram_tensor("Wv", [F, A], F32, kind="ExternalInput")[:],
        nc.dram_tensor("Wproj", [A, F], F32, kind="ExternalInput")[:],
        nc.dram_tensor("bproj", [1, F], F32, kind="ExternalInput")[:],
        nc.dram_tensor("out", [B_LOCAL, F], F32, kind="ExternalOutput")[:],
    )


_BUILT = {}


def _get_nc(collective=True, reps=1, upto=10):
    key = (collective, reps, upto)
    if key not in _BUILT:
        nc = bacc.Bacc("TRN2", target_bir_lowering=False, debug=False,
                       num_devices=N_CORES)
        with tile.TileContext(nc) as tc:
            io = _declare_io(nc)
            for r in range(reps):
                if r:
                    tc.strict_bb_all_engine_barrier()
                _emit(tc, collective=collective, io=io, upto=upto)
        nc.compile()
        _BUILT[key] = nc
    return _BUILT[key]


def _make_in_maps(inputs):
    features = np.ascontiguousarray(np.asarray(inputs["features"],
                                               dtype=np.float32))
    labels = np.ascontiguousarray(np.asarray(inputs["labels"])).astype(np.int64)
    Wq = np.ascontiguousarray(np.asarray(inputs["Wq"], dtype=np.float32))
    Wk = np.ascontiguousarray(np.asarray(inputs["Wk"], dtype=np.float32))
    Wv = np.ascontiguousarray(np.asarray(inputs["Wv"], dtype=np.float32))
    Wproj = np.ascontiguousarray(np.asarray(inputs["Wproj"], dtype=np.float32))
    bproj = np.ascontiguousarray(
        np.asarray(inputs["bproj"], dtype=np.float32)).reshape(1, F)

    in_maps = []
    for cix in range(N_CORES):
        fl = features[cix * B_LOCAL:(cix + 1) * B_LOCAL]
        ll = labels[cix * B_LOCAL:(cix + 1) * B_LOCAL]
        lab2d = np.ascontiguousarray(
            ll.astype(np.float32).reshape(NB, P).T)
        in_maps.append({
            "features": fl,
            "labels_f32": lab2d,
            "Wq": Wq, "Wk": Wk, "Wv": Wv, "Wproj": Wproj, "bproj": bproj,
        })
    return in_maps


def _assemble(inputs, results):
    features = np.asarray(inputs["features"], dtype=np.float32)
    out = np.empty((N_CORES * B_LOCAL, 2 * F), np.float32)
    out[:, :F] = features
    for cix in range(N_CORES):
        out[cix * B_LOCAL:(cix + 1) * B_LOCAL, F:] = results[cix]["out"]
    return out


def _run(inputs, **run_kwargs):
    nc = _get_nc()
    in_maps = _make_in_maps(inputs)
    res = run_bass_kernel_spmd(nc, in_maps, list(range(N_CORES)), **run_kwargs)
    return _assemble(inputs, res.results), res


def kernel(**inputs):
    out, _ = _run(inputs)
    return out
